# revision 14
# baseline (speedup 1.0000x reference)
"""Trainium2 Bass kernel for a GPT-J-style (parallel-residual) decoder layer.

Problem: B=2, S=2048, D=1024, H=16 heads x 64, rotary_dim=16, FF=4096, causal.

Sharding (8 NeuronCores): data-parallel over batch (2) x tensor-parallel over
heads/FFN (4).  Core c handles batch c//4 and TP rank r=c%4: heads 4r..4r+3
(256 of the 1024 attention dims), FFN rows 1024r..1024r+1024.
LayerNorm affine params are folded into the weights on the host; the device
computes one normalized activation xhat shared by attention and FFN.

v3 strategy:
 - All transposes (xhat, q, k) are PE `is_transpose` matmuls (no DRAM staging).
 - QKV and probs@V run in fp8e4 with DoubleRow (2 K-tiles/pass = 2x PE);
   FFN1/FFN2/Wo stay bf16 (fp8 there costs ~1.7e-2 rel err - too much).
 - FFN2 partials staged in SBUF (no DRAM round trip); output written in bf16.
   Host sums the 4 TP partials per batch and adds x + b_o + b2.
"""

import numpy as np
import ml_dtypes

import concourse.bass as bass
import concourse.mybir as mybir
import concourse.tile as tile
import concourse.bass_utils as bass_utils
from concourse import bacc
from concourse.bass import ds, ts

B, S, D = 2, 2048, 1024
H, HD = 16, 64
ROT, RH = 16, 8
FF = 4096
EPS = 1e-5
P = 128
NT = S // P            # 16 sequence tiles
NPAIR = NT // 2        # 8 sequence-tile pairs (fp8 DoubleRow AV)
DC = D // P            # 8 model-dim chunks
NH = 4                 # heads per core
DSH = NH * HD          # 256 attention dims per core
FSH = FF // 4          # 1024 FFN rows per core
NCORES = 8
WSC = 64.0             # fp8 weight pre-scale

QKV8 = True            # fp8 DoubleRow QKV projection
AV8 = True             # fp8 DoubleRow probs @ V

F32 = mybir.dt.float32
BF16 = mybir.dt.bfloat16
F8 = mybir.dt.float8e4
AF = mybir.ActivationFunctionType
ALU = mybir.AluOpType
DR = mybir.MatmulPerfMode.DoubleRow
bf16 = ml_dtypes.bfloat16
f8 = ml_dtypes.float8_e4m3

PTD = F8 if AV8 else BF16


def _body(tc, aps, gelu_func):
    nc = tc.nc
    x_d = aps["x"]
    out_d = aps["outp"].rearrange("(c p) s -> c p s", p=P)   # [8, 128, 2048]

    with (
        tc.tile_pool(name="const", bufs=1) as const,
        tc.tile_pool(name="big", bufs=1) as big,
        tc.tile_pool(name="xp", bufs=3) as xp,
        tc.tile_pool(name="statp", bufs=4) as statp,
        tc.tile_pool(name="xhp", bufs=6) as xhp,
        tc.tile_pool(name="rotp", bufs=3) as rotp,
        tc.tile_pool(name="ptp", bufs=2) as ptp,
        tc.tile_pool(name="sump", bufs=2) as sump,
        tc.tile_pool(name="obp", bufs=2) as obp,
        tc.tile_pool(name="wstp", bufs=4) as wstp,
    ):
        # ---- persistent SBUF ----
        def cin(name, shape, dtype):
            t = const.tile(list(shape), dtype, name=f"c_{name}")
            nc.sync.dma_start(t[:], aps[name])
            return t

        wqkv_sb = cin("wqkv", [P, 4, 2, 3 * DSH], F8) if QKV8 else \
            cin("wqkv", [P, DC, 3 * DSH], BF16)
        bqkv_sb = cin("bqkv", [P, 3 * DSH], F32)
        wo_sb = cin("wo", [P, 2, D], BF16)
        w1_d = aps["w1"]      # [pd, ft, cd, pf] - streamed per ft
        w2_d = aps["w2"]      # [pf, et, cf, pd] - streamed per et
        b1_sb = cin("b1p", [P, DC], F32)
        cos_sb = cin("cosr", [P, NT, RH], BF16)
        sin_sb = cin("sinr", [P, NT, RH], BF16)
        mask_sb = cin("maskd", [P, P], PTD)
        id_sb = cin("ident", [P, P], BF16)
        eps_sb = const.tile([P, 1], F32)
        nc.vector.memset(eps_sb[:], EPS)
        ones_sb = const.tile([1, HD], BF16)
        nc.vector.memset(ones_sb[:], 1.0)

        vp = big.tile([P, NT, NH, HD + 4], PTD)  # pad: pair stride %16==0     # v token-major + ones col
        qe = big.tile([P, 2, S], BF16)              # q e-major
        ke = big.tile([P, 2, S], BF16)              # k e-major
        ot = big.tile([P, 2, S], BF16)              # attn out (normalized)
        hid = big.tile([P, DC, S], BF16)            # ffn hidden, f-major

        xt_pool = tc.alloc_tile_pool(name="xtp", bufs=1)
        xhatT = xt_pool.tile([P, DC, S], BF16)      # xhat dim-major [d, s]
        qk_pool = tc.alloc_tile_pool(name="qkp", bufs=1)
        qk = qk_pool.tile([P, NT, 2 * DSH], BF16)   # q,k token-major
        if QKV8:
            xhatT8 = qk_pool.tile([P, DC, S], F8)

        nc.gpsimd.memset(vp[:, :, :, HD:HD + 1], 1.0)

        # ---- Phase A+B: LayerNorm -> PE transpose -> QKV, per 512-tok group
        with (
            tc.tile_pool(name="tps", bufs=3, space="PSUM") as tps,
            tc.tile_pool(name="qaps", bufs=2, space="PSUM") as qaps,
            tc.tile_pool(name="qbps", bufs=2, space="PSUM") as qbps,
        ):
            for g in range(4):
                xh = []
                for tt in range(4):
                    t = 4 * g + tt
                    x_t = xp.tile([P, D], BF16, tag="xt")
                    nc.sync.dma_start(x_t[:], x_d[ts(t, P), :])
                    st = statp.tile([P, 2, 6], F32, tag="st")
                    xr = x_t[:].rearrange("p (a f) -> p a f", f=512)
                    for sg in range(2):
                        nc.vector.bn_stats(st[:, sg, :], xr[:, sg, :])
                    mv = statp.tile([P, 2], F32, tag="mv")
                    nc.vector.bn_aggr(mv[:], st[:])
                    std = statp.tile([P, 1], F32, tag="sd")
                    nc.scalar.activation(std[:], mv[:, 1:2], AF.Sqrt,
                                         bias=eps_sb[:])
                    rstd = statp.tile([P, 1], F32, tag="rs")
                    nc.vector.reciprocal(rstd[:], std[:])
                    xt = xhp.tile([P, D], BF16, tag="xh")
                    nc.vector.tensor_scalar(out=xt[:], in0=x_t[:],
                                            scalar1=mv[:, 0:1],
                                            scalar2=rstd[:],
                                            op0=ALU.subtract, op1=ALU.mult)
                    xh.append(xt)
                # PE transpose: xh (token-major) -> xhatT[:, c, 512g:512g+512]
                for c in range(DC):
                    pst = tps.tile([P, 512], BF16, tag="tp")
                    for tt in range(4):
                        nc.tensor.transpose(pst[:, ts(tt, P)],
                                            xh[tt][:, ts(c, P)], id_sb[:])
                    nc.vector.tensor_copy(out=xhatT[:, c, ts(g, 512)],
                                          in_=pst[:])
                    if QKV8:
                        nc.scalar.activation(xhatT8[:, c, ts(g, 512)],
                                             pst[:], AF.Copy)
                # QKV for this group's tiles
                for tt in range(4):
                    t = 4 * g + tt
                    psa = qaps.tile([P, 512], F32, tag="qa")
                    psb = qbps.tile([P, 256], F32, tag="qb")
                    if QKV8:
                        for pr in range(4):
                            l = xhatT8[:, ds(2 * pr, 2), ts(t, P)]
                            nc.tensor.matmul(psa[:], lhsT=l,
                                             rhs=wqkv_sb[:, pr, :, 0:512],
                                             start=(pr == 0),
                                             stop=(pr == 3), perf_mode=DR)
                            nc.tensor.matmul(psb[:], lhsT=l,
                                             rhs=wqkv_sb[:, pr, :, 512:768],
                                             start=(pr == 0),
                                             stop=(pr == 3), perf_mode=DR)
                        nc.vector.scalar_tensor_tensor(
                            out=qk[:, t, :], in0=psa[:], scalar=1.0 / WSC,
                            in1=bqkv_sb[:, 0:512], op0=ALU.mult, op1=ALU.add)
                        nc.vector.scalar_tensor_tensor(
                            out=vp[:, t, :, 0:HD],
                            in0=psb[:].rearrange("p (h e) -> p h e", h=NH),
                            scalar=1.0 / WSC,
                            in1=bqkv_sb[:, 512:768].rearrange(
                                "p (h e) -> p h e", h=NH),
                            op0=ALU.mult, op1=ALU.add)
                    else:
                        for c in range(DC):
                            l = xhatT[:, c, ts(t, P)]
                            nc.tensor.matmul(psa[:], lhsT=l,
                                             rhs=wqkv_sb[:, c, 0:512],
                                             start=(c == 0),
                                             stop=(c == DC - 1))
                            nc.tensor.matmul(psb[:], lhsT=l,
                                             rhs=wqkv_sb[:, c, 512:768],
                                             start=(c == 0),
                                             stop=(c == DC - 1))
                        nc.vector.tensor_tensor(out=qk[:, t, :], in0=psa[:],
                                                in1=bqkv_sb[:, 0:512],
                                                op=ALU.add)
                        nc.vector.tensor_tensor(
                            out=vp[:, t, :, 0:HD],
                            in0=psb[:].rearrange("p (h e) -> p h e", h=NH),
                            in1=bqkv_sb[:, 512:768].rearrange(
                                "p (h e) -> p h e", h=NH), op=ALU.add)

            # rotary on q and k (token-major, in place) - gpsimd (SBUF only)
            cosb = cos_sb[:].unsqueeze(2).to_broadcast([P, NT, NH, RH])
            sinb = sin_sb[:].unsqueeze(2).to_broadcast([P, NT, NH, RH])
            for part in range(2):   # 0: q, 1: k
                sl = qk[:, :, ds(DSH * part, DSH)].rearrange(
                    "p t (h e) -> p t h e", h=NH)
                x1 = sl[:, :, :, 0:RH]
                x2 = sl[:, :, :, RH:ROT]
                t1 = rotp.tile([P, NT, NH, RH], BF16, tag="rt")
                t2 = rotp.tile([P, NT, NH, RH], BF16, tag="rt")
                t3 = rotp.tile([P, NT, NH, RH], BF16, tag="rt")
                nc.gpsimd.tensor_tensor(out=t1[:], in0=x1, in1=cosb,
                                        op=ALU.mult)
                nc.gpsimd.tensor_tensor(out=t2[:], in0=x2, in1=sinb,
                                        op=ALU.mult)
                nc.gpsimd.tensor_tensor(out=t1[:], in0=t1[:], in1=t2[:],
                                        op=ALU.subtract)
                nc.gpsimd.tensor_tensor(out=t2[:], in0=x1, in1=sinb,
                                        op=ALU.mult)
                nc.gpsimd.tensor_tensor(out=t3[:], in0=x2, in1=cosb,
                                        op=ALU.mult)
                nc.gpsimd.tensor_tensor(out=t2[:], in0=t2[:], in1=t3[:],
                                        op=ALU.add)
                nc.gpsimd.tensor_copy(out=x1, in_=t1[:])
                nc.gpsimd.tensor_copy(out=x2, in_=t2[:])

            # PE transpose q,k -> e-major qe/ke
            for c4 in range(4):          # q0 q1 k0 k1 e-chunks
                dstt = qe if c4 < 2 else ke
                for g in range(4):
                    pst = tps.tile([P, 512], BF16, tag="tp")
                    for tt in range(4):
                        t = 4 * g + tt
                        nc.tensor.transpose(pst[:, ts(tt, P)],
                                            qk[:, t, ts(c4, P)], id_sb[:])
                    if (c4 + g) % 2 == 0:
                        nc.vector.tensor_copy(out=dstt[:, c4 % 2, ts(g, 512)],
                                              in_=pst[:])
                    else:
                        nc.scalar.activation(dstt[:, c4 % 2, ts(g, 512)],
                                             pst[:], AF.Copy)

        # ---- FFN1 (bf16) -> GELU -> hid ----
        with tc.tile_pool(name="ff1ps", bufs=2, space="PSUM") as ff1ps:
            for ft in range(DC):
                w1t = wstp.tile([P, DC, P], BF16, tag="wst",
                                name=f"w1t_{ft}")
                nc.sync.dma_start(w1t[:], w1_d[:, ft])
                for half in range(2):
                    psf = ff1ps.tile([P, 2, 512], F32, tag="ff1")
                    for c in range(DC):
                        for q in range(2):
                            sc = 2 * half + q
                            nc.tensor.matmul(
                                psf[:, q, :], lhsT=w1t[:, c, :],
                                rhs=xhatT[:, c, ts(sc, 512)],
                                start=(c == 0), stop=(c == DC - 1))
                    for q in range(2):
                        nc.scalar.activation(
                            hid[:, ft, ds(1024 * half + 512 * q, 512)],
                            psf[:, q, :], gelu_func,
                            bias=b1_sb[:, ft:ft + 1])

        qk_pool.release()   # qk + xhatT8 dead
        xt_pool.release()   # xhatT dead; space reused by ff2p
        ff2_pool = tc.alloc_tile_pool(name="ff2p", bufs=1)
        ff2p = ff2_pool.tile([P, DC, S], BF16)      # FFN2 partials [d, s]

        # ---- Phase C: attention, FFN2 interleaved ----
        ff2_deck = [[0, 1], [2, 3], [4, 5], [6, 7]]
        with (
            tc.tile_pool(name="scps", bufs=1, space="PSUM") as scps,
            tc.tile_pool(name="ovps", bufs=4, space="PSUM") as ovps,
            tc.tile_pool(name="rbps", bufs=1, space="PSUM") as rbps,
            tc.tile_pool(name="f2ps", bufs=1, space="PSUM") as f2ps,
        ):
            def emit_ff2(et):
                w2t = wstp.tile([P, DC, P], BF16, tag="wst",
                                name=f"w2t_{et}")
                nc.gpsimd.dma_start(w2t[:], w2_d[:, et])
                for sc in range(4):
                    ps = f2ps.tile([P, 512], F32, tag="f2",
                                   name=f"f2_{et}_{sc}")
                    for c in range(DC):
                        nc.tensor.matmul(ps[:], lhsT=w2t[:, c, :],
                                         rhs=hid[:, c, ts(sc, 512)],
                                         start=(c == 0), stop=(c == DC - 1))
                    if sc % 2 == 0:
                        nc.vector.tensor_copy(out=ff2p[:, et, ts(sc, 512)],
                                              in_=ps[:])
                    else:
                        nc.scalar.activation(ff2p[:, et, ts(sc, 512)],
                                             ps[:], AF.Copy)

            def emit_head(h):
                base = HD * (h % 2)
                cix = h // 2
                ov = [ovps.tile([HD + 1, 512], F32, tag="ov",
                                name=f"ov_{h}_{i}") for i in range(4)]
                pt = ptp.tile([P, 2, S], PTD, tag="pt", name=f"pt_{h}")
                for pi in range(NPAIR):
                    for j in range(2):
                        i = 2 * pi + j
                        ncols = S - P * i
                        nb = (ncols + 1023) // 1024
                        for bi in range(nb):
                            w = min(1024, ncols - 1024 * bi)
                            st = scps.tile([P, 2, 512], F32, tag="sc",
                                           name=f"sc_{h}_{i}_{bi}")
                            for q in range(2):
                                wq = min(512, w - 512 * q)
                                if wq <= 0:
                                    break
                                nc.tensor.matmul(
                                    st[:, q, 0:wq],
                                    lhsT=ke[base:base + HD, cix, ts(i, P)],
                                    rhs=qe[base:base + HD, cix,
                                           ds(P * i + 1024 * bi + 512 * q,
                                              wq)],
                                    start=True, stop=True)
                                nc.scalar.activation(
                                    pt[:, j, ds(P * i + 1024 * bi + 512 * q,
                                                wq)],
                                    st[:, q, 0:wq], AF.Exp, scale=0.125)
                        # causal mask on the diagonal 128 cols
                        nc.gpsimd.tensor_tensor(
                            out=pt[:, j, ts(i, P)], in0=pt[:, j, ts(i, P)],
                            in1=mask_sb[:], op=ALU.mult)
                    # zero the gap of the odd tile (cols 256pi..256pi+128)
                    nc.gpsimd.memset(pt[:, 1, ds(256 * pi, P)], 0.0)
                    # probs @ V
                    for sc in range(pi // 2, 4):
                        lo = max(512 * sc, 256 * pi)
                        wid = 512 * (sc + 1) - lo
                        if AV8:
                            nc.tensor.matmul(
                                ov[sc][:, ds(lo - 512 * sc, wid)],
                                lhsT=vp[:, ds(2 * pi, 2), h, 0:HD + 1],
                                rhs=pt[:, :, ds(lo, wid)],
                                start=(pi == 0),
                                stop=(pi == min(NPAIR - 1, 2 * sc + 1)),
                                perf_mode=DR)
                        else:
                            for j in range(2):
                                i = 2 * pi + j
                                lo2 = max(lo, P * i)
                                wid2 = 512 * (sc + 1) - lo2
                                nc.tensor.matmul(
                                    ov[sc][:, ds(lo2 - 512 * sc, wid2)],
                                    lhsT=vp[:, i, h, 0:HD + 1],
                                    rhs=pt[:, j, ds(lo2, wid2)],
                                    start=(i == 0),
                                    stop=(i == min(NT - 1, 4 * sc + 3)))
                # normalization: ot = ov * (1/rowsum)
                rinvs = []
                for sc in range(4):
                    dst = ot[base:base + HD, cix, ts(sc, 512)]
                    nc.vector.tensor_copy(out=dst, in_=ov[sc][0:HD, :])
                    sume = sump.tile([1, 512], F32, tag="se",
                                     name=f"se_{h}_{sc}")
                    nc.vector.tensor_copy(out=sume[:],
                                          in_=ov[sc][HD:HD + 1, :])
                    rinv = sump.tile([1, 512], F32, tag="ri",
                                     name=f"ri_{h}_{sc}")
                    nc.vector.reciprocal_approx_fast(out=rinv[:], in_=sume[:])
                    rinv_bf = sump.tile([1, 512], BF16, tag="rib",
                                        name=f"rib_{h}_{sc}")
                    nc.vector.tensor_copy(out=rinv_bf[:], in_=rinv[:])
                    rinvs.append(rinv_bf)
                for sc in range(4):
                    rbp = rbps.tile([HD, 512], F32, tag="rb",
                                    name=f"rb_{h}_{sc}")
                    nc.tensor.matmul(rbp[:], lhsT=ones_sb[:],
                                     rhs=rinvs[sc][:], start=True, stop=True)
                    dst = ot[base:base + HD, cix, ts(sc, 512)]
                    nc.vector.tensor_tensor(out=dst, in0=dst, in1=rbp[:],
                                            op=ALU.mult)

            for h in range(NH):
                emit_head(h)
                for et in ff2_deck[h]:
                    emit_ff2(et)

        # ---- Phase D: Wo + add FFN2 partials, write out ----
        with tc.tile_pool(name="wops", bufs=4, space="PSUM") as wops:
            for et in range(DC):
                ob = obp.tile([P, 4, 512], BF16, tag="ob", name=f"ob_{et}")
                for sc in range(4):
                    po = wops.tile([P, 512], F32, tag="wo")
                    for c in range(2):
                        nc.tensor.matmul(po[:], lhsT=wo_sb[:, c, ts(et, P)],
                                         rhs=ot[:, c, ts(sc, 512)],
                                         start=(c == 0), stop=(c == 1))
                    nc.vector.tensor_tensor(
                        out=ob[:, sc, :], in0=po[:],
                        in1=ff2p[:, et, ts(sc, 512)], op=ALU.add)
                eng = nc.sync if et % 2 == 0 else nc.gpsimd
                eng.dma_start(out_d[et], ob[:].rearrange("p a b -> p (a b)"))
        ff2_pool.release()


def build(gelu_func=None):
    if gelu_func is None:
        gelu_func = AF.Gelu
    nc = bacc.Bacc("TRN2", target_bir_lowering=False, debug=False,
                   enable_asserts=True, num_devices=NCORES)
    aps = {}

    def din(name, shape, dtype):
        aps[name] = nc.dram_tensor(name, list(shape), dtype,
                                   kind="ExternalInput").ap()

    din("x", (S, D), BF16)
    if QKV8:
        din("wqkv", (P, 4, 2, 3 * DSH), F8)
    else:
        din("wqkv", (P, DC, 3 * DSH), BF16)
    din("bqkv", (P, 3 * DSH), F32)
    din("wo", (P, 2, D), BF16)
    din("w1", (P, DC, DC, P), BF16)
    din("b1p", (P, DC), F32)
    din("w2", (P, DC, DC, P), BF16)
    din("cosr", (P, NT, RH), BF16)
    din("sinr", (P, NT, RH), BF16)
    din("maskd", (P, P), F8 if AV8 else BF16)
    din("ident", (P, P), BF16)
    aps["outp"] = nc.dram_tensor("outp", [D, S], BF16,
                                 kind="ExternalOutput").ap()

    with tile.TileContext(nc) as tc:
        _body(tc, aps, gelu_func)
    nc.compile()
    return nc


def make_in_maps(inputs):
    x = np.asarray(inputs["x"], np.float32)
    Wqkv = np.asarray(inputs["W_qkv"], np.float32)
    b_qkv = np.asarray(inputs["b_qkv"], np.float32)
    Wo = np.asarray(inputs["W_o"], np.float32)
    ln1w = np.asarray(inputs["ln1_w"], np.float32)
    ln1b = np.asarray(inputs["ln1_b"], np.float32)
    ln2w = np.asarray(inputs["ln2_w"], np.float32)
    ln2b = np.asarray(inputs["ln2_b"], np.float32)
    W1 = np.asarray(inputs["W1"], np.float32)
    b1 = np.asarray(inputs["b1"], np.float32)
    W2 = np.asarray(inputs["W2"], np.float32)
    freqs = np.asarray(inputs["freqs_cis"], np.float32)

    cos = freqs[0, 0, :, :, 0]
    sin = freqs[0, 0, :, :, 1]
    cosr = np.ascontiguousarray(
        cos.reshape(NT, P, RH).transpose(1, 0, 2)).astype(bf16)
    sinr = np.ascontiguousarray(
        sin.reshape(NT, P, RH).transpose(1, 0, 2)).astype(bf16)
    kq = np.arange(P)
    maskd = (kq[:, None] <= kq[None, :]).astype(f8 if AV8 else bf16)
    ident = np.eye(P, dtype=np.float32).astype(bf16)

    in_maps = []
    for core in range(NCORES):
        b = core // 4
        r = core % 4
        sl = slice(256 * r, 256 * r + 256)
        Ws = np.concatenate([Wqkv[0:D][sl], Wqkv[D:2 * D][sl],
                             Wqkv[2 * D:3 * D][sl]], 0)          # [768, 1024]
        bq = np.concatenate([b_qkv[0:D][sl], b_qkv[D:2 * D][sl],
                             b_qkv[2 * D:3 * D][sl]], 0)
        Wsp = Ws * ln1w[None, :]
        bqp = (bq + Ws @ ln1b).astype(np.float32)
        if QKV8:
            # wqkv_l[pd, pair, two, e] = Wsp[e, (2*pair+two)*128+pd] * WSC
            wqkv_l = np.ascontiguousarray(
                (Wsp.T * WSC).reshape(4, 2, P, 3 * DSH).transpose(
                    2, 0, 1, 3)).astype(f8)
        else:
            wqkv_l = np.ascontiguousarray(
                Wsp.T.reshape(DC, P, 3 * DSH).transpose(1, 0, 2)).astype(bf16)
        bqkv_l = np.ascontiguousarray(
            np.broadcast_to(bqp[None, :], (P, 3 * DSH))).astype(np.float32)
        Wos = Wo[:, sl]                                          # [1024, 256]
        wo_l = np.ascontiguousarray(
            Wos.T.reshape(2, P, D).transpose(1, 0, 2)).astype(bf16)
        W1s = W1[FSH * r: FSH * (r + 1)]                         # [1024, 1024]
        W1p = W1s * ln2w[None, :]
        b1p = (b1[FSH * r: FSH * (r + 1)] + W1s @ ln2b).astype(np.float32)
        # w1_l[pd, ft, cd, pf] = W1p[ft*128+pf, cd*128+pd]
        w1_l = np.ascontiguousarray(
            W1p.reshape(DC, P, DC, P).transpose(3, 0, 2, 1)).astype(bf16)
        b1_l = np.ascontiguousarray(b1p.reshape(DC, P).T).astype(np.float32)
        W2s = W2[:, FSH * r: FSH * (r + 1)]                      # [1024, 1024]
        # w2_l[pf, et, cf, pd] = W2s[et*128+pd, cf*128+pf]
        w2_l = np.ascontiguousarray(
            W2s.reshape(DC, P, DC, P).transpose(3, 0, 2, 1)).astype(bf16)
        in_maps.append(dict(
            x=np.ascontiguousarray(x[b]).astype(bf16), wqkv=wqkv_l,
            bqkv=bqkv_l, wo=wo_l, w1=w1_l, b1p=b1_l, w2=w2_l,
            cosr=cosr, sinr=sinr, maskd=maskd, ident=ident))
    return in_maps


def gather(inputs, results):
    x = np.asarray(inputs["x"], np.float32)
    bias = (np.asarray(inputs["b_o"], np.float32)
            + np.asarray(inputs["b2"], np.float32))
    outs = [np.asarray(res["outp"], np.float32) for res in results]
    out = np.empty((B, S, D), np.float32)
    for b in range(B):
        acc = outs[4 * b] + outs[4 * b + 1] + outs[4 * b + 2] + outs[4 * b + 3]
        out[b] = x[b] + acc.T + bias[None, :]
    return out


_CACHE = {}


def kernel(**inputs):
    if "nc" not in _CACHE:
        _CACHE["nc"] = build()
    nc = _CACHE["nc"]
    in_maps = make_in_maps(inputs)
    res = bass_utils.run_bass_kernel_spmd(nc, in_maps,
                                          core_ids=list(range(NCORES)))
    return gather(inputs, res.results)
